# revision 5
# baseline (speedup 1.0000x reference)
#!/usr/bin/env python3
"""2-layer GAT on 8 NeuronCores (Bass/Tile) — v2.

Sharding: nodes partitioned across 8 cores by dst id (graph parallel) with a
degree-balanced assignment (snake-LPT over 784 (core, chunk) bins) so the
per-(chunk, window) edge-cell sizes are tight across cores.

Layer-1 node features are stored fp8e4m3 (h1) + fp16 (attention logits) in
256B-stride table rows, allgathered once, and fetched per edge with 144B
dma_gather descriptors.  Gathers are batched per (group-of-10-chunks, window)
to amortize the SWDGE fixed overhead.  Per-edge math runs (c,h)-major so the
softmax-weighted feature scaling hits the DVE 2x mode.  Segment softmax +
aggregation use indicator matmuls on the tensor engine.
"""
import sys
import numpy as np

sys.path.insert(0, "/opt/pypackages")
sys.path.insert(0, "/opt/trn_rl_repo")

import concourse.bass as bass
import concourse.bacc as bacc
import concourse.tile as tile
import concourse.mybir as mybir
from concourse.bass_utils import run_bass_kernel_spmd

# problem constants
N = 100000
F_IN = 512
NHID = 16
HEADS = 8
NCLASS = 40
NEG_SLOPE = 0.2

NCORES = 8
NCH = 98                      # dst chunks per core (128 dsts each)
NPAD = NCH * 128              # 12544 padded rows per core shard
NBINS = NCORES * NCH          # 784 (core, chunk) bins
NW = 4                        # gather windows (= core pairs)
WROWS = 2 * NPAD              # 25088 table rows per window (int16-safe)
GSZ = 5                       # chunks per gather group

ROW1B = 256   # tab1 row stride bytes: [h1 f8 x128 | asrc f16 x8 | adst f16 x8]
G1E = 144     # gathered bytes per tab1 row (h1 + asrc)
ROW2 = 128    # tab2 row stride (f16 elems): [h2 40 | asrc2 | adst2 | pad]
G2E = 41      # gathered f16 elems per tab2 row (h2 + asrc2)
GAB = 16      # gathered bytes per adst fetch (tab1 cols 144:160)

F16 = mybir.dt.float16
F32 = mybir.dt.float32
F8 = mybir.dt.float8e4
I16 = mybir.dt.int16


def _wrap_block(v):
    """Wrap a 1-D int16 stream (len % 16 == 0) into dma_gather idx layout
    [16, L/16], replicated to 128 partitions."""
    w = v.reshape(-1, 16).T
    return np.tile(w, (8, 1))


def _dma_gather_raw(gp, out_ap, in_ap, idxs_ap, num_idxs, elem_size, elem_step,
                    queue_num=0):
    """dma_gather allowing elem_size (bytes read per row) that is not a
    multiple of 256B; the table row stride (elem_step) still must be."""
    from concourse.bass import exact_div
    stride_bytes = elem_step * mybir.dt.size(in_ap.dtype)
    stride_bytes_256 = exact_div(stride_bytes, 256)
    _in_ap = gp.lower_ap_dma(in_ap, for_custom_bir_dma=True)
    _idxs_ap = gp.lower_ap(idxs_ap)
    _out_ap = gp.lower_ap(out_ap)
    return gp.add_instruction(
        mybir.InstDMAGatherAnt(
            name=gp.bass.get_next_instruction_name(),
            ins=[*_in_ap, _idxs_ap, gp.lower_val_access(gp.to_reg(num_idxs))],
            outs=[_out_ap],
            transpose=False, num_idxs=num_idxs, elem_size=elem_size,
            stride_bytes_256=stride_bytes_256, gen_mode=0,
            single_packet=False, queue_num=queue_num,
            sbuf_tokens_per_rank=0, sbuf_free_dim_per_rank=0,
            sbuf_free_dim_pad_per_rank=0, sbuf_byte_offset=0))


def _groups():
    gs = []
    d = 0
    while d < NCH:
        gs.append((d, min(d + GSZ, NCH)))
        d += GSZ
    return gs


def _layout(shapes):
    """Stream layout from the static cell shapes [NCH, NW].

    Returns (cell_off [NCH, NW] stream offsets, group list of dicts with
    goff/gt/spans/segs)."""
    cell_off = np.zeros((NCH, NW), dtype=np.int64)
    groups = []
    pos = 0
    for (dlo, dhi) in _groups():
        goff = pos
        spans = []
        for w in range(NW):
            s0 = pos
            for d in range(dlo, dhi):
                cell_off[d, w] = pos
                pos += int(shapes[d, w])
            spans.append((s0 - goff, pos - s0))
        segs = []
        for d in range(dlo, dhi):
            segs.append([(int(cell_off[d, w] - goff), int(shapes[d, w]))
                         for w in range(NW)])
        groups.append(dict(goff=goff, gt=(pos - goff) // 128, dlo=dlo,
                           dhi=dhi, spans=spans, segs=segs))
    return cell_off, groups, pos


def _prep(x, edge_index, W1, att_src1, att_dst1, W2, att_src2, att_dst2,
          b1=None, b2=None):
    """Host-side sharding/packing. Returns (in_maps, shapes, node_core,
    node_l)."""
    x = np.asarray(x, np.float32)
    # self-loops are handled analytically on-device (they are core-local);
    # only the real edges go through the gather stream.
    src = np.asarray(edge_index[0])
    dst = np.asarray(edge_index[1])

    # degree-balanced snake assignment of nodes into (core, chunk, slot)
    deg = np.bincount(dst, minlength=N)
    order = np.argsort(-deg, kind="stable")
    rank = np.empty(N, dtype=np.int64)
    rank[order] = np.arange(N)
    p = rank // NBINS
    j = rank % NBINS
    jj = np.where(p % 2 == 0, j, NBINS - 1 - j)
    node_core = (jj % NCORES).astype(np.int64)
    node_chunk = jj // NCORES
    node_slot = p                      # < 128 since N/NBINS < 128
    node_l = node_chunk * 128 + node_slot

    # per-edge quantities
    kd = node_core[dst]
    dd = node_chunk[dst]
    sd = node_slot[dst]
    ld = node_l[dst]
    ks = node_core[src]
    sloc = (ks & 1) * NPAD + node_l[src]   # row within window [0, 25088)
    w = ks >> 1

    # cell shapes (max over cores, rounded to 128)
    cellid = (kd * NCH + dd) * NW + w
    counts = np.bincount(cellid, minlength=NCORES * NCH * NW)
    counts = counts.reshape(NCORES, NCH, NW)
    shapes = (np.ceil(counts.max(axis=0) / 128.0) * 128).astype(np.int64)

    cell_off, groups, L = _layout(shapes)
    t_total = L // 128

    # edge positions: sort by (core, cell stream offset, sloc); rank in cell
    co = cell_off[dd, w]
    okey = np.lexsort((sloc, co, kd))
    kd_s, co_s, sloc_s, ld_s, sd_s = (a[okey] for a in (kd, co, sloc, ld, sd))
    cid_s = kd_s * L + co_s            # unique per (core, cell)
    first = np.concatenate([[True], cid_s[1:] != cid_s[:-1]])
    idx_first = np.flatnonzero(first)
    erank = np.arange(len(cid_s)) - np.repeat(idx_first, np.diff(
        np.concatenate([idx_first, [len(cid_s)]])))
    pos = co_s + erank

    idx1 = np.zeros((NCORES, L), dtype=np.int16)
    idxd = np.zeros((NCORES, L), dtype=np.int16)
    dstloc = np.full((NCORES, L), 255.0, dtype=np.float16)
    idx1[kd_s, pos] = sloc_s.astype(np.int16)
    idxd[kd_s, pos] = ld_s.astype(np.int16)
    dstloc[kd_s, pos] = sd_s.astype(np.float16)

    IDX1 = np.stack([_wrap_block(idx1[k]) for k in range(NCORES)])
    IDXD = np.stack([_wrap_block(idxd[k]) for k in range(NCORES)])
    DSTLOC = dstloc.reshape(NCORES, t_total, 128).transpose(0, 2, 1).copy()
    # fp8 one-hot indicators for phase E: INDF8[k][p, t*128 + s] = (dstloc==s)
    import ml_dtypes
    INDF8 = np.zeros((NCORES, 128, t_total * 128), dtype=ml_dtypes.float8_e4m3)
    tpos = pos // 128
    ppos = pos % 128
    INDF8[kd_s, ppos, tpos * 128 + sd_s.astype(np.int64)] = 1.0

    # weights, (c,h)-major feature ordering for layer 1 outputs
    asrc1 = np.asarray(att_src1, np.float64).reshape(HEADS, NHID)
    adst1 = np.asarray(att_dst1, np.float64).reshape(HEADS, NHID)
    W1r = np.asarray(W1, np.float64).reshape(F_IN, HEADS, NHID)
    W1ch = W1r.transpose(0, 2, 1).reshape(F_IN, HEADS * NHID)   # (c,h)-major
    W1as = np.einsum("khc,hc->kh", W1r, asrc1)
    W1ad = np.einsum("khc,hc->kh", W1r, adst1)
    W1ext = np.concatenate([W1ch, W1as, W1ad], axis=1).astype(np.float16)

    W2 = np.asarray(W2, np.float64)
    W2as = W2 @ np.asarray(att_src2, np.float64).reshape(NCLASS, 1)
    W2ad = W2 @ np.asarray(att_dst2, np.float64).reshape(NCLASS, 1)
    W2e = np.concatenate([W2, W2as, W2ad], axis=1)              # [128, 42]
    W2ext = W2e.reshape(HEADS, NHID, NCLASS + 2).transpose(1, 0, 2).reshape(
        HEADS * NHID, NCLASS + 2).astype(np.float16)            # rows (c,h)

    b1v = (np.zeros(HEADS * NHID) if b1 is None else np.asarray(b1, np.float64))
    b1ch = b1v.reshape(HEADS, NHID).T.reshape(1, HEADS * NHID).astype(np.float32)
    b2v = (np.zeros(NCLASS) if b2 is None else np.asarray(b2, np.float64))
    b2v = b2v.reshape(1, NCLASS).astype(np.float32)

    in_maps = []
    for k in range(NCORES):
        sel = node_core == k
        Xp = np.zeros((NPAD, F_IN), dtype=np.float16)
        Xp[node_l[sel]] = x[sel].astype(np.float16)
        in_maps.append({
            "xT": np.ascontiguousarray(Xp.T),
            "W1ext": W1ext,
            "W2ext": W2ext,
            "IDX1": IDX1[k],
            "IDXD": IDXD[k],
            "DSTLOC": DSTLOC[k],
            "INDF8": INDF8[k],
            "B1": b1ch,
            "B2": b2v,
        })
    return in_maps, shapes, node_core, node_l


def _build(shapes):
    from concourse.masks import make_identity
    _, groups, L = _layout(shapes)
    t_total = L // 128

    nc = bacc.Bacc("TRN2", target_bir_lowering=False, debug=False,
                   enable_asserts=False, num_devices=NCORES,
                   num_swdge_queues=4)

    xT = nc.dram_tensor("xT", [F_IN, NPAD], F16, kind="ExternalInput")
    W1e = nc.dram_tensor("W1ext", [F_IN, 144], F16, kind="ExternalInput")
    W2e = nc.dram_tensor("W2ext", [128, NCLASS + 2], F16, kind="ExternalInput")
    IDX1 = nc.dram_tensor("IDX1", [128, t_total * 8], I16, kind="ExternalInput")
    IDXD = nc.dram_tensor("IDXD", [128, t_total * 8], I16, kind="ExternalInput")
    DSTLOC = nc.dram_tensor("DSTLOC", [128, t_total], F16, kind="ExternalInput")
    INDF8 = nc.dram_tensor("INDF8", [128, t_total * 128], F8, kind="ExternalInput")
    B1 = nc.dram_tensor("B1", [1, 128], F32, kind="ExternalInput")
    B2 = nc.dram_tensor("B2", [1, NCLASS], F32, kind="ExternalInput")
    OUT = nc.dram_tensor("out", [NPAD, NCLASS], F32, kind="ExternalOutput")

    tab1_sh = nc.dram_tensor("tab1_sh", [NPAD, ROW1B], F8, kind="Internal")
    tab1 = nc.dram_tensor("tab1", [NPAD * NCORES, ROW1B], F8, kind="Internal",
                          addr_space="Shared")
    tab2_sh = nc.dram_tensor("tab2_sh", [NPAD, ROW2], F16, kind="Internal")
    tab2 = nc.dram_tensor("tab2", [NPAD * NCORES, ROW2], F16, kind="Internal",
                          addr_space="Shared")

    eq = mybir.AluOpType.is_equal
    mult = mybir.AluOpType.mult
    amax = mybir.AluOpType.max
    aadd = mybir.AluOpType.add
    sub = mybir.AluOpType.subtract
    AF = mybir.ActivationFunctionType

    with tile.TileContext(nc) as tc:
        _phase_a(nc, tc, xT, W1e, tab1_sh)
        nc.gpsimd.collective_compute(
            "AllGather", mybir.AluOpType.bypass,
            replica_groups=[list(range(NCORES))],
            ins=[tab1_sh[:]], outs=[tab1[:]])
        _phase_c(nc, tc, groups, make_identity, IDX1, IDXD, INDF8, B1, W2e,
                 tab1, tab1_sh, tab2_sh, eq, mult, amax, aadd, AF)
        nc.gpsimd.collective_compute(
            "AllGather", mybir.AluOpType.bypass,
            replica_groups=[list(range(NCORES))],
            ins=[tab2_sh[:]], outs=[tab2[:]])
        _phase_e(nc, tc, groups, IDX1, IDXD, INDF8, B2, tab2, tab2_sh, OUT,
                 eq, mult, amax, aadd, sub, AF)

    nc.compile()
    return nc


def _phase_a(nc, tc, xT, W1e, tab1_sh):
    with tc.tile_pool(name="sbA", bufs=1) as sba, \
         tc.tile_pool(name="sbA2", bufs=8) as sba2, \
         tc.tile_pool(name="psA", bufs=6, space="PSUM") as psa:
        xts = [sba.tile([128, NPAD], F16, tag=f"xt{k}", name=f"xt{k}")
               for k in range(4)]
        w1s = [sba.tile([128, 144], F16, tag=f"w1{k}", name=f"w1{k}")
               for k in range(4)]
        for k in range(4):
            nc.sync.dma_start(xts[k][:], xT[k * 128:(k + 1) * 128, :])
            nc.sync.dma_start(w1s[k][:], W1e[k * 128:(k + 1) * 128, :])
        BA = 7  # chunks per batched store
        for nb in range((NCH + BA - 1) // BA):
            lo, hi = nb * BA, min((nb + 1) * BA, NCH)
            nbc = hi - lo
            h8 = sba2.tile([128, BA * 128], F8, tag="h8", name="h8")
            a16 = sba2.tile([128, BA * 16], F16, tag="a16", name="a16")
            for i, nt in enumerate(range(lo, hi)):
                ps = psa.tile([128, 144], F32, tag="psA", name="psA")
                for k in range(4):
                    nc.tensor.matmul(ps[:],
                                     lhsT=xts[k][:, nt * 128:(nt + 1) * 128],
                                     rhs=w1s[k][:], start=(k == 0), stop=(k == 3))
                nc.vector.tensor_copy(h8[:, i * 128:(i + 1) * 128], ps[:, 0:128])
                nc.vector.tensor_copy(a16[:, i * 16:(i + 1) * 16], ps[:, 128:144])
            nc.sync.dma_start(
                tab1_sh[lo * 128:hi * 128, 0:128]
                .rearrange("(t p) e -> p t e", p=128),
                h8[:, 0:nbc * 128].rearrange("p (t e) -> p t e", e=128))
            nc.sync.dma_start(
                tab1_sh[lo * 128:hi * 128, 128:160].bitcast(F16)
                .rearrange("(t p) e -> p t e", p=128),
                a16[:, 0:nbc * 16].rearrange("p (t e) -> p t e", e=16))


def _phase_c(nc, tc, groups, make_identity, IDX1, IDXD, INDF8, B1, W2e,
             tab1, tab1_sh, tab2_sh, eq, mult, amax, aadd, AF):
    GTM = max(g["gt"] for g in groups)
    with tc.tile_pool(name="sbC", bufs=1) as sbc, \
         tc.tile_pool(name="preC", bufs=4) as pre, \
         tc.tile_pool(name="indC", bufs=3) as ip, \
         tc.tile_pool(name="mainC", bufs=2) as mn, \
         tc.tile_pool(name="smC", bufs=6) as sm, \
         tc.tile_pool(name="psC", bufs=3, space="PSUM") as psc, \
         tc.tile_pool(name="psC2", bufs=2, space="PSUM") as psc2:
        ident = sbc.tile([128, 128], F16, tag="ident", name="ident")
        make_identity(nc, ident[:])
        w2s = sbc.tile([128, NCLASS + 2], F16, tag="w2s", name="w2s")
        nc.sync.dma_start(w2s[:], W2e[:])
        b1t = sbc.tile([128, 128], F32, tag="b1t", name="b1t")
        nc.sync.dma_start(b1t[:], B1[:].to_broadcast([128, 128]))

        def _pre_c(gi, g):
            goff, GT = g["goff"] // 128, g["gt"]
            i1 = pre.tile([128, GTM * 8], I16, tag="i1", name="i1")
            nc.sync.dma_start(i1[:, 0:GT * 8], IDX1[:, goff * 8:(goff + GT) * 8])
            idd = pre.tile([128, GTM * 8], I16, tag="idd", name="idd")
            nc.sync.dma_start(idd[:, 0:GT * 8], IDXD[:, goff * 8:(goff + GT) * 8])
            ga = pre.tile([128, GTM * GAB], F8, tag="ga", name="ga")
            _dma_gather_raw(nc.gpsimd,
                            ga[:, 0:GT * GAB].rearrange("p (t e) -> p t e", e=GAB),
                            tab1_sh[:, 144:160], idd[:, 0:GT * 8], GT * 128,
                            GAB, ROW1B, queue_num=gi % 4)

            ind = ip.tile([128, GTM * 128], F8, tag="ind", name="ind")
            nc.sync.dma_start(ind[:, 0:GT * 128],
                              INDF8[:, goff * 128:(goff + GT) * 128])
            return i1, ga, ind

        PF = 2  # groups prefetched ahead (fills the AllGather window)
        fetched = [_pre_c(gj, groups[gj]) for gj in range(min(PF + 1, len(groups)))]
        for gi, g in enumerate(groups):
            goff, GT = g["goff"] // 128, g["gt"]
            i1, ga, ind = fetched[gi]
            # --- AG-dependent main work ---
            g1 = mn.tile([128, GTM * G1E], F8, tag="g1", name="g1")
            for w in range(NW):
                s0, sl = g["spans"][w]
                if sl == 0:
                    continue
                _dma_gather_raw(
                    nc.gpsimd,
                    g1[:, (s0 // 128) * G1E:((s0 + sl) // 128) * G1E]
                    .rearrange("p (t e) -> p t e", e=G1E),
                    tab1[w * WROWS:(w + 1) * WROWS, :],
                    i1[:, s0 // 16:(s0 + sl) // 16], sl, G1E, ROW1B,
                    queue_num=w)

            att = mn.tile([128, GTM * 8], F16, tag="att", name="att")
            at3 = att[:, 0:GT * 8].rearrange("p (t h) -> p t h", h=8)
            nc.vector.tensor_tensor(
                out=at3,
                in0=g1[:, 0:GT * G1E].rearrange("p (t e) -> p t e", e=G1E)
                [:, :, 128:144].bitcast(F16),
                in1=ga[:, 0:GT * GAB].rearrange("p (t e) -> p t e", e=GAB)
                .bitcast(F16),
                op=aadd)
            nc.vector.scalar_tensor_tensor(
                out=at3, in0=at3, scalar=NEG_SLOPE, in1=at3, op0=mult, op1=amax)
            wst = mn.tile([128, GTM * 8], F16, tag="wst", name="wst")
            nc.scalar.activation(out=wst[:, 0:GT * 8], in_=att[:, 0:GT * 8],
                                 func=AF.Exp)

            # ust in fp8 (matches the fp8 indicator lhsT); h1 is already fp8
            ust = mn.tile([128, GTM * 136], F8, tag="ust", name="ust")
            us4 = ust[:, 0:GT * 136].rearrange("p (t e) -> p t e", e=136)
            nc.vector.tensor_tensor(
                out=us4[:, :, 0:128].rearrange("p t (c h) -> p t c h", h=8),
                in0=g1[:, 0:GT * G1E].rearrange("p (t e) -> p t e", e=G1E)
                [:, :, 0:128].rearrange("p t (c h) -> p t c h", h=8),
                in1=wst[:, 0:GT * 8].rearrange("p (t o h) -> p t o h", o=1, h=8)
                .to_broadcast([128, GT, NHID, 8]),
                op=mult)
            nc.vector.tensor_copy(
                us4[:, :, 128:136],
                wst[:, 0:GT * 8].rearrange("p (t h) -> p t h", h=8))

            # batched self-loop contribution for the group (core-local)
            nd = g["dhi"] - g["dlo"]
            hlr = mn.tile([128, GSZ * 160], F8, tag="hlr", name="hlr")
            nc.sync.dma_start(
                hlr[:, 0:nd * 160].rearrange("p (t e) -> p t e", e=160),
                tab1_sh[g["dlo"] * 128:g["dhi"] * 128, 0:160]
                .rearrange("(t p) e -> p t e", p=128))
            hl3 = hlr[:, 0:nd * 160].rearrange("p (t e) -> p t e", e=160)
            atl = mn.tile([128, GSZ * 8], F16, tag="atl", name="atl")
            al3 = atl[:, 0:nd * 8].rearrange("p (t h) -> p t h", h=8)
            nc.vector.tensor_tensor(out=al3, in0=hl3[:, :, 128:144].bitcast(F16),
                                    in1=hl3[:, :, 144:160].bitcast(F16), op=aadd)
            nc.vector.scalar_tensor_tensor(
                out=al3, in0=al3, scalar=NEG_SLOPE, in1=al3, op0=mult, op1=amax)
            exl = mn.tile([128, GSZ * 8], F16, tag="exl", name="exl")
            nc.scalar.activation(out=exl[:, 0:nd * 8], in_=atl[:, 0:nd * 8],
                                 func=AF.Exp)
            snm = mn.tile([128, GSZ * 128], F16, tag="snm", name="snm")
            sn3 = snm[:, 0:nd * 128].rearrange("p (t e) -> p t e", e=128)
            nc.scalar.activation(out=sn3, in_=hl3[:, :, 0:128], func=AF.Copy)
            nc.vector.tensor_tensor(
                out=sn3.rearrange("p t (c h) -> p t c h", h=8),
                in0=sn3.rearrange("p t (c h) -> p t c h", h=8),
                in1=exl[:, 0:nd * 8].rearrange("p (t o h) -> p t o h", o=1, h=8)
                .to_broadcast([128, nd, NHID, 8]),
                op=mult)

            for d in range(g["dlo"], g["dhi"]):
                di = d - g["dlo"]
                segs = g["segs"][di]
                tiles = [(s // 128 + t)
                         for (s, ln) in segs for t in range(ln // 128)]
                ps1 = psc.tile([128, 136], F32, tag="ps1", name="ps1")
                for i, t in enumerate(tiles):
                    nc.tensor.matmul(ps1[:], lhsT=ind[:, t * 128:(t + 1) * 128],
                                     rhs=ust[:, t * 136:(t + 1) * 136],
                                     start=(i == 0), stop=(i == len(tiles) - 1))

                rows = slice(d * 128, (d + 1) * 128)
                den = sm.tile([128, 8], F32, tag="den", name="den")
                nc.vector.tensor_tensor(out=den[:], in0=ps1[:, 128:136],
                                        in1=exl[:, di * 8:(di + 1) * 8], op=aadd)
                rc = sm.tile([128, 8], F32, tag="rc", name="rc")
                nc.vector.reciprocal(rc[:], den[:])
                o1 = sm.tile([128, 128], F32, tag="o1", name="o1")
                nc.vector.tensor_tensor(out=o1[:], in0=ps1[:, 0:128],
                                        in1=snm[:, di * 128:(di + 1) * 128],
                                        op=aadd)
                nc.vector.tensor_tensor(
                    out=o1[:].rearrange("p (c h) -> p c h", h=8),
                    in0=o1[:].rearrange("p (c h) -> p c h", h=8),
                    in1=rc[:].rearrange("p (o h) -> p o h", o=1)
                    .to_broadcast([128, NHID, 8]),
                    op=mult)
                nc.vector.tensor_tensor(out=o1[:], in0=o1[:], in1=b1t[:], op=aadd)
                # elu = max(x,0) + (exp(min(x,0)) - 1)
                t1 = sm.tile([128, 128], F16, tag="t1", name="t1")
                nc.vector.tensor_scalar_min(t1[:], o1[:], 0.0)
                t2 = sm.tile([128, 128], F16, tag="t2", name="t2")
                nc.scalar.activation(out=t2[:], in_=t1[:], func=AF.Exp)
                nc.vector.tensor_scalar_max(t1[:], o1[:], 0.0)
                elu = sm.tile([128, 128], F16, tag="elu", name="elu")
                nc.vector.scalar_tensor_tensor(
                    out=elu[:], in0=t2[:], scalar=-1.0, in1=t1[:],
                    op0=aadd, op1=aadd)

                psT = psc2.tile([128, 128], F16, tag="psT", name="psT")
                nc.tensor.transpose(psT[:], elu[:], ident[:])
                eluT = sm.tile([128, 128], F16, tag="eluT", name="eluT")
                nc.scalar.activation(out=eluT[:], in_=psT[:], func=AF.Copy)
                ps2a = psc2.tile([128, NCLASS + 2], F32, tag="ps2a", name="ps2a")
                nc.tensor.matmul(ps2a[:], lhsT=eluT[:], rhs=w2s[:],
                                 start=True, stop=True)

                h2r = sm.tile([128, NCLASS + 2], F16, tag="h2r", name="h2r")
                nc.scalar.activation(out=h2r[:], in_=ps2a[:, 0:NCLASS + 2],
                                     func=AF.Copy)
                nc.sync.dma_start(tab2_sh[rows, 0:NCLASS + 2], h2r[:])
            if gi + PF + 1 < len(groups):
                fetched.append(_pre_c(gi + PF + 1, groups[gi + PF + 1]))


def _phase_e(nc, tc, groups, IDX1, IDXD, INDF8, B2, tab2, tab2_sh, OUT,
             eq, mult, amax, aadd, sub, AF):
    GTM = max(g["gt"] for g in groups)
    with tc.tile_pool(name="sbE", bufs=1) as sbe, \
         tc.tile_pool(name="preE", bufs=5) as pre, \
         tc.tile_pool(name="indE", bufs=5) as ip, \
         tc.tile_pool(name="mainE", bufs=2) as mn, \
         tc.tile_pool(name="smE", bufs=4) as sm, \
         tc.tile_pool(name="lgE", bufs=GSZ + 2) as lgp, \
         tc.tile_pool(name="psE", bufs=4, space="PSUM") as pse:
        b2t = sbe.tile([128, NCLASS], F32, tag="b2t", name="b2t")
        nc.sync.dma_start(b2t[:], B2[:].to_broadcast([128, NCLASS]))

        def _pre_e(gi, g):
            goff, GT = g["goff"] // 128, g["gt"]
            i1 = pre.tile([128, GTM * 8], I16, tag="i1e", name="i1e")
            nc.sync.dma_start(i1[:, 0:GT * 8], IDX1[:, goff * 8:(goff + GT) * 8])
            idd = pre.tile([128, GTM * 8], I16, tag="idde", name="idde")
            nc.sync.dma_start(idd[:, 0:GT * 8], IDXD[:, goff * 8:(goff + GT) * 8])
            ga2 = pre.tile([128, GTM * 8], F16, tag="ga2", name="ga2")
            _dma_gather_raw(nc.gpsimd,
                            ga2[:, 0:GT * 8].rearrange("p (t e) -> p t e", e=8),
                            tab2_sh[:, 40:48], idd[:, 0:GT * 8], GT * 128,
                            8, ROW2, queue_num=gi % 4)

            ind = ip.tile([128, GTM * 128], F8, tag="inde", name="inde")
            nc.sync.dma_start(ind[:, 0:GT * 128],
                              INDF8[:, goff * 128:(goff + GT) * 128])
            return i1, ga2, ind

        PFE = 3
        fetched = [_pre_e(gj, groups[gj]) for gj in range(min(PFE + 1, len(groups)))]
        for gi, g in enumerate(groups):
            goff, GT = g["goff"] // 128, g["gt"]
            i1, ga2, ind = fetched[gi]

            g2 = mn.tile([128, GTM * G2E], F16, tag="g2", name="g2")
            for w in range(NW):
                s0, sl = g["spans"][w]
                if sl == 0:
                    continue
                _dma_gather_raw(
                    nc.gpsimd,
                    g2[:, (s0 // 128) * G2E:((s0 + sl) // 128) * G2E]
                    .rearrange("p (t e) -> p t e", e=G2E),
                    tab2[w * WROWS:(w + 1) * WROWS, :],
                    i1[:, s0 // 16:(s0 + sl) // 16], sl, G2E, ROW2,
                    queue_num=w)

            g23 = g2[:, 0:GT * G2E].rearrange("p (t e) -> p t e", e=G2E)
            at2 = mn.tile([128, GTM], F16, tag="at2", name="at2")
            at23 = at2[:, 0:GT].rearrange("p (t h) -> p t h", h=1)
            nc.vector.tensor_tensor(
                out=at23, in0=g23[:, :, NCLASS:NCLASS + 1],
                in1=ga2[:, 0:GT * 8].rearrange("p (t e) -> p t e", e=8)
                [:, :, 1:2], op=aadd)
            nc.vector.scalar_tensor_tensor(
                out=at23, in0=at23, scalar=NEG_SLOPE, in1=at23,
                op0=mult, op1=amax)
            w2t = mn.tile([128, GTM], F16, tag="w2t", name="w2t")
            nc.scalar.activation(out=w2t[:, 0:GT], in_=at2[:, 0:GT], func=AF.Exp)

            # gw = [h2 * w | w] in fp8 (matches the fp8 indicator lhsT)
            gw = mn.tile([128, GTM * G2E], F8, tag="gw", name="gw")
            gw3 = gw[:, 0:GT * G2E].rearrange("p (t e) -> p t e", e=G2E)
            nc.vector.tensor_tensor(
                out=gw3[:, :, 0:NCLASS],
                in0=g23[:, :, 0:NCLASS],
                in1=w2t[:, 0:GT].rearrange("p (t s) -> p t s", s=1)
                .to_broadcast([128, GT, NCLASS]),
                op=mult)
            nc.vector.tensor_copy(
                gw3[:, :, NCLASS:NCLASS + 1],
                w2t[:, 0:GT].rearrange("p (t s) -> p t s", s=1))

            # batched self-loop contribution for the group (core-local)
            nd = g["dhi"] - g["dlo"]
            h2l = mn.tile([128, GSZ * 42], F16, tag="h2l", name="h2l")
            nc.sync.dma_start(
                h2l[:, 0:nd * 42].rearrange("p (t e) -> p t e", e=42),
                tab2_sh[g["dlo"] * 128:g["dhi"] * 128, 0:42]
                .rearrange("(t p) e -> p t e", p=128))
            h23 = h2l[:, 0:nd * 42].rearrange("p (t e) -> p t e", e=42)
            atle = mn.tile([128, GSZ], F16, tag="atle", name="atle")
            ae3 = atle[:, 0:nd].rearrange("p (t h) -> p t h", h=1)
            nc.vector.tensor_tensor(out=ae3, in0=h23[:, :, 40:41],
                                    in1=h23[:, :, 41:42], op=aadd)
            nc.vector.scalar_tensor_tensor(
                out=ae3, in0=ae3, scalar=NEG_SLOPE, in1=ae3, op0=mult, op1=amax)
            w2l = mn.tile([128, GSZ], F32, tag="w2l", name="w2l")
            nc.scalar.activation(out=w2l[:, 0:nd], in_=atle[:, 0:nd], func=AF.Exp)
            s2g = mn.tile([128, GSZ * NCLASS], F32, tag="s2g", name="s2g")
            nc.vector.tensor_tensor(
                out=s2g[:, 0:nd * NCLASS].rearrange("p (t e) -> p t e", e=NCLASS),
                in0=h23[:, :, 0:NCLASS],
                in1=w2l[:, 0:nd].rearrange("p (t s) -> p t s", s=1)
                .to_broadcast([128, nd, NCLASS]),
                op=mult)

            smG = lgp.tile([128, GSZ], F32, tag="smG", name="smG")
            lgs = []
            for d in range(g["dlo"], g["dhi"]):
                di = d - g["dlo"]
                segs = g["segs"][di]
                tiles = [(s // 128 + t)
                         for (s, ln) in segs for t in range(ln // 128)]
                ps2 = pse.tile([128, NCLASS + 1], F32, tag="ps2", name="ps2")
                for i, t in enumerate(tiles):
                    nc.tensor.matmul(ps2[:], lhsT=ind[:, t * 128:(t + 1) * 128],
                                     rhs=gw[:, t * G2E:t * G2E + NCLASS + 1],
                                     start=(i == 0), stop=(i == len(tiles) - 1))

                dn2 = sm.tile([128, 1], F32, tag="dn2", name="dn2")
                nc.vector.tensor_tensor(out=dn2[:], in0=ps2[:, NCLASS:NCLASS + 1],
                                        in1=w2l[:, di:di + 1], op=aadd)
                rc2 = sm.tile([128, 1], F32, tag="rc2", name="rc2")
                nc.vector.reciprocal(rc2[:], dn2[:])
                o2 = sm.tile([128, NCLASS], F32, tag="o2", name="o2")
                nc.vector.tensor_tensor(out=o2[:], in0=ps2[:, 0:NCLASS],
                                        in1=s2g[:, di * NCLASS:(di + 1) * NCLASS],
                                        op=aadd)
                lg = lgp.tile([128, NCLASS], F32, tag="lg", name="lg")
                nc.vector.scalar_tensor_tensor(out=lg[:], in0=o2[:],
                                               scalar=rc2[:], in1=b2t[:],
                                               op0=mult, op1=aadd)
                ex = sm.tile([128, NCLASS], F32, tag="ex", name="ex")
                nc.scalar.activation(out=ex[:], in_=lg[:], func=AF.Exp,
                                     accum_out=smG[:, d - g["dlo"]:d - g["dlo"] + 1])
                lgs.append((d, lg))

            # one Ln per group (avoids act-table thrash), then finalize
            lsG = lgp.tile([128, GSZ], F32, tag="lsG", name="lsG")
            nd = g["dhi"] - g["dlo"]
            nc.scalar.activation(out=lsG[:, 0:nd], in_=smG[:, 0:nd], func=AF.Ln)
            for d, lg in lgs:
                i = d - g["dlo"]
                fin = sm.tile([128, NCLASS], F32, tag="fin", name="fin")
                nc.vector.tensor_scalar(out=fin[:], in0=lg[:],
                                        scalar1=lsG[:, i:i + 1],
                                        scalar2=None, op0=sub)
                nc.sync.dma_start(OUT[d * 128:(d + 1) * 128, :], fin[:])
            if gi + PFE + 1 < len(groups):
                fetched.append(_pre_e(gi + PFE + 1, groups[gi + PFE + 1]))


_CACHE = {}


def kernel(x, edge_index, W1, att_src1, att_dst1, b1, W2, att_src2, att_dst2, b2):
    in_maps, shapes, node_core, node_l = _prep(
        np.asarray(x), np.asarray(edge_index), np.asarray(W1),
        np.asarray(att_src1), np.asarray(att_dst1), np.asarray(W2),
        np.asarray(att_src2), np.asarray(att_dst2), b1=b1, b2=b2)
    key = shapes.tobytes()
    if key not in _CACHE:
        _CACHE[key] = _build(shapes)
    nc = _CACHE[key]
    res = run_bass_kernel_spmd(nc, in_maps, core_ids=list(range(NCORES)))
    per_core = np.stack([res.results[k]["out"] for k in range(NCORES)])
    out = per_core[node_core, node_l]        # [N, NCLASS]
    return out.astype(np.float32)
